# revision 2
# baseline (speedup 1.0000x reference)
"""Trainium2 Bass kernel for nn_CategorySpecificInitNet (moe_routing).

kernel(**inputs) takes the FULL unsharded inputs (keys as in
reference.setup_inputs()) and returns the FULL [B, 128] float32 output.

Strategy — expert-parallel (dispatch-by-category), as in the proven
baseline, plus an fp8 DoubleRow fast path for the two big encoder
layers:

  - rows are stably sorted by cat_idx; core k gets category k's rows,
    zero-padded to a static capacity (max category count rounded up to
    256);
  - encoder L3 (256->256 linear) is constant-folded into decoder layer 1
    on the host (exact algebra);
  - L1 (768->512) and L2 (512->256) run as fp8e4m3 DoubleRow matmuls
    with full hi/lo error compensation: both weights and activations are
    split x ~= x_hi + x_lo (each e4m3), and y = W@x is computed as
    (Wh+Wl)@x_hi + Wh@x_lo (the Wl@x_lo term, ~0.15% rms, is dropped).
    DoubleRow processes two 128-contraction slots per instruction at
    half the per-row cost, so the 3-term compensated product runs at
    0.75x the fp32r cycle count while keeping end-to-end max rel error
    ~3e-3 (measured against the fp32 reference).
  - feature hi/lo splitting happens on the host (it is a dtype-level
    repacking of the input, like casting to bf16); a1's hi/lo pair is
    produced on device by ACT relu->f32 + DVE cast + DVE subtract.
  - the decoder (256->256->256->128) stays in float32r.
  - all compute is feature-major [features(partitions), rows(free)];
    per-core row tiles of 512 (first/odd tail tiles of 256); the decoder
    is software-pipelined one tile behind the encoder.
"""
import sys

for _p in ("/opt/trn_rl_repo",):
    if _p not in sys.path:
        sys.path.append(_p)

import numpy as np
import ml_dtypes

import concourse.bass as bass
import concourse.bacc as bacc
import concourse.mybir as mybir
import concourse.tile as tile
from concourse import bass_utils

FR = mybir.dt.float32r
F32 = mybir.dt.float32
F8 = mybir.dt.float8e4
E4NP = ml_dtypes.float8_e4m3
DR = mybir.MatmulPerfMode.DoubleRow
Alu = mybir.AluOpType
ActF = mybir.ActivationFunctionType

B, C, H1, H2, HO = 32768, 768, 512, 256, 256
DH, LAT, K = 256, 128, 8
N_CORES = 8
TILE = 512

# fp8 scales (powers of two).  features absmax ~5.1 -> x16 = ~81 (<240
# e4m3 max); a1 absmax ~2.8 -> x32 = ~90; weight absmax ~0.1 -> x1024.
SF = 16.0
SA1 = 32.0
SW1 = 1024.0
SW2 = 1024.0
A1_SCALE = SA1 / (SW1 * SF)       # psum1 -> a1-units scale for ACT
A2_SCALE = 1.0 / (SW2 * SA1)      # psum2 -> true a2 units

# bias_all columns: be1*SA1 [4] | be2 [2] | bf [2] | bd2 [2] | bd3 [1]
OB1, OB2, OD1, OD2, OD3 = 0, 4, 6, 8, 10
NBIAS = 11

nC, nH1, nH2, nDH = C // 128, H1 // 128, H2 // 128, DH // 128


def _build_nc(cap, tile_n=512):
    assert cap % 256 == 0
    ntile = cap // tile_n
    tiles = [tile_n] * ntile
    rem = cap - ntile * tile_n
    # first tile small (fast PE start), tail tile small (short drain)
    if rem >= 256:
        tiles.insert(0, 256)
        rem -= 256
    if rem:
        tiles.append(rem)
    # if cap is an exact multiple of 512, split first and last tiles to 256
    if tiles[0] == tile_n:
        tiles[0] = 256
        tiles.append(256)
    offs = [sum(tiles[:i]) for i in range(len(tiles))]
    nt = len(tiles)
    nc = bacc.Bacc(name="catnet_fp8")

    # fp8 hi/lo features, packed (2, C, cap): [0]=hi, [1]=lo
    fhl = nc.dram_tensor("fhl", (2, C, cap), F8, kind="ExternalInput")
    # L1 weights: wa1[p, c, i, m]: i=0,1 both hold Wh (DoubleRow slot pair
    # (Wh,xh),(Wh,xl)); wb1[p, cc, i, m]: (Wl_{2cc}, Wl_{2cc+1})
    wa1 = nc.dram_tensor("wa1", (128, nC, 2, H1), F8, kind="ExternalInput")
    wb1 = nc.dram_tensor("wb1", (128, nC // 2, 2, H1), F8, kind="ExternalInput")
    wa2 = nc.dram_tensor("wa2", (128, nH1, 2, H2), F8, kind="ExternalInput")
    wb2 = nc.dram_tensor("wb2", (128, nH1 // 2, 2, H2), F8, kind="ExternalInput")
    wd1 = nc.dram_tensor("wd1", (H2, DH), FR, kind="ExternalInput")  # We3 @ Wd1
    wd2 = nc.dram_tensor("wd2", (DH, DH), FR, kind="ExternalInput")
    wd3 = nc.dram_tensor("wd3", (DH, LAT), FR, kind="ExternalInput")
    bias_all = nc.dram_tensor("bias_all", (128, NBIAS), F32, kind="ExternalInput")
    out = nc.dram_tensor("out", (LAT, cap), F32, kind="ExternalOutput")

    with tile.TileContext(nc) as tc:
        with (
            tc.tile_pool(name="wp", bufs=1) as wp,
            tc.tile_pool(name="fp", bufs=3) as fp,
            tc.tile_pool(name="xp", bufs=2) as xp,
            tc.tile_pool(name="ap", bufs=3) as ap,
            tc.tile_pool(name="dp", bufs=2) as dp,
            tc.tile_pool(name="ps_w", bufs=6, space="PSUM") as ps_w,
            tc.tile_pool(name="ps_o", bufs=2, space="PSUM") as ps_o,
        ):
            bias_t = wp.tile([128, NBIAS], F32, tag="bias")
            nc.gpsimd.dma_start(bias_t[:], bias_all[:])
            wa1_t = wp.tile([128, nC, 2, H1], F8, tag="wa1")
            # first L1 c-blocks land first so tile-0 matmuls can start early
            nc.gpsimd.dma_start(wa1_t[:, :2, :, :], wa1[:, :2, :, :])
            nc.gpsimd.dma_start(wa1_t[:, 2:, :, :], wa1[:, 2:, :, :])
            wb1_t = wp.tile([128, nC // 2, 2, H1], F8, tag="wb1")
            nc.gpsimd.dma_start(wb1_t[:], wb1[:])
            wa2_t = wp.tile([128, nH1, 2, H2], F8, tag="wa2")
            nc.gpsimd.dma_start(wa2_t[:], wa2[:])
            wb2_t = wp.tile([128, nH1 // 2, 2, H2], F8, tag="wb2")
            nc.gpsimd.dma_start(wb2_t[:], wb2[:])
            wd1_t = wp.tile([128, nH2, DH], FR, tag="wd1")
            nc.gpsimd.dma_start(wd1_t[:], wd1.rearrange("(c p) d -> p c d", p=128))
            wd2_t = wp.tile([128, nDH, DH], FR, tag="wd2")
            nc.gpsimd.dma_start(wd2_t[:], wd2.rearrange("(c p) d -> p c d", p=128))
            wd3_t = wp.tile([128, nDH, LAT], FR, tag="wd3")
            nc.gpsimd.dma_start(wd3_t[:], wd3.rearrange("(c p) d -> p c d", p=128))

            fhl_r = fhl.rearrange("t (c p) b -> p t c b", p=128)

            def fp8_layer3t(wa_t, wb_t, x_t, ncb, m, tn, pw):
                """3-term compensated fp8 DoubleRow matmul accumulation.

                wa_t: [128, ncb, 2, M] (Wh dup), wb_t: [128, ncb/2, 2, M]
                x_t:  [128, 2, ncb, tn] hi/lo pair
                accumulates (Wh+Wl)@xh + Wh@xl into pw[:, :tn].
                """
                msl = bass.ts(m, 128)
                for ch in range(0, tn, 256):
                    n = min(256, tn - ch)
                    sl = slice(ch, ch + n)
                    first = True
                    for c in range(ncb):
                        nc.tensor.matmul(
                            pw[:, sl], wa_t[:, c, :, msl], x_t[:, :, c, sl],
                            start=first, stop=False, perf_mode=DR)
                        first = False
                    for cc in range(ncb // 2):
                        nc.tensor.matmul(
                            pw[:, sl], wb_t[:, cc, :, msl],
                            x_t[:, 0, 2 * cc:2 * cc + 2, sl],
                            start=False, stop=(cc == ncb // 2 - 1), perf_mode=DR)

            def emit_enc(t):
                tn = tiles[t]
                sl = slice(offs[t], offs[t] + tn)
                fx_fl = fp.tile([128, 2, nC, tile_n], F8, tag="fx")
                fx = fx_fl[:, :, :, :tn]
                eng = nc.sync if t % 2 == 0 else nc.gpsimd
                eng.dma_start(fx[:], fhl_r[:, :, :, sl])

                # L1: 768 -> 512, fp8 3-term
                x32_fl = xp.tile([128, nH1, tile_n], F32, tag="x32")
                x32 = x32_fl[:, :, :tn]
                for m in range(nH1):
                    pw_fl = ps_w.tile([128, tile_n], F32, tag="pw")
                    pw = pw_fl[:, :tn]
                    fp8_layer3t(wa1_t, wb1_t, fx, nC, m, tn, pw_fl)
                    nc.scalar.activation(x32[:, m, :], pw[:], ActF.Relu,
                                         bias=bias_t[:, OB1 + m:OB1 + m + 1],
                                         scale=A1_SCALE)
                # a1 hi/lo pair (DVE): hi = cast(x32), lo = x32 - hi
                a1_fl = xp.tile([128, 2, nH1, tile_n], F8, tag="a1")
                a1 = a1_fl[:, :, :, :tn]
                nc.vector.tensor_copy(a1[:, 0], x32[:])
                nc.vector.tensor_tensor(a1[:, 1], x32[:], a1[:, 0], Alu.subtract)

                # L2: 512 -> 256, fp8 3-term
                a2 = []
                for m in range(nH2):
                    pw_fl = ps_w.tile([128, tile_n], F32, tag="pw")
                    pw = pw_fl[:, :tn]
                    fp8_layer3t(wa2_t, wb2_t, a1, nH1, m, tn, pw_fl)
                    x_fl = ap.tile([128, tile_n], FR, tag=f"a2_{m}")
                    x = x_fl[:, :tn]
                    nc.scalar.activation(x[:], pw[:], ActF.Relu,
                                         bias=bias_t[:, OB2 + m:OB2 + m + 1],
                                         scale=A2_SCALE)
                    a2.append(x)
                return a2

            def emit_d1(t, h):
                tn = tiles[t]
                d1 = []
                for m in range(nDH):
                    pw_fl = ps_w.tile([128, tile_n], F32, tag="pw")
                    pw = pw_fl[:, :tn]
                    for c in range(nH2):
                        nc.tensor.matmul(pw[:], wd1_t[:, c, bass.ts(m, 128)], h[c][:],
                                         start=(c == 0), stop=(c == nH2 - 1))
                    x_fl = dp.tile([128, tile_n], FR, tag=f"d1_{m}")
                    x = x_fl[:, :tn]
                    if m % 2 == 1:
                        nc.vector.tensor_scalar(x[:], pw[:],
                                                bias_t[:, OD1 + m:OD1 + m + 1],
                                                0.0, Alu.add, Alu.max)
                    else:
                        nc.scalar.activation(x[:], pw[:], ActF.Relu,
                                             bias=bias_t[:, OD1 + m:OD1 + m + 1])
                    d1.append(x)
                return d1

            def emit_d2_d3_store(t, d1):
                tn = tiles[t]
                d2 = []
                for m in range(nDH):
                    pw_fl = ps_w.tile([128, tile_n], F32, tag="pw")
                    pw = pw_fl[:, :tn]
                    for c in range(nDH):
                        nc.tensor.matmul(pw[:], wd2_t[:, c, bass.ts(m, 128)], d1[c][:],
                                         start=(c == 0), stop=(c == nDH - 1))
                    x_fl = dp.tile([128, tile_n], FR, tag=f"d2_{m}")
                    x = x_fl[:, :tn]
                    bb = bias_t[:, OD2 + m:OD2 + m + 1]
                    if m % 2 == 0:
                        nc.vector.tensor_scalar(x[:], pw[:], bb, 0.0, Alu.add, Alu.max)
                    else:
                        nc.scalar.activation(x[:], pw[:], ActF.Relu, bias=bb)
                    d2.append(x)
                po_fl = ps_o.tile([128, tile_n], F32, tag="out")
                po = po_fl[:, :tn]
                for c in range(nDH):
                    nc.tensor.matmul(po[:], wd3_t[:, c, :], d2[c][:],
                                     start=(c == 0), stop=(c == nDH - 1))
                osb_fl = ap.tile([128, tile_n], F32, tag="osb")
                osb = osb_fl[:, :tn]
                nc.scalar.activation(osb[:], po[:], ActF.Identity,
                                     bias=bias_t[:, OD3:OD3 + 1])
                nc.gpsimd.dma_start(out[:, offs[t]:offs[t] + tn], osb[:])

            # decoder one tile behind the encoder
            pend = None
            for t in range(nt):
                h = emit_enc(t)
                if pend is not None:
                    emit_d2_d3_store(pend[0], pend[1])
                d1 = emit_d1(t, h)
                pend = (t, d1)
            emit_d2_d3_store(pend[0], pend[1])

    nc.finalize()
    return nc


def _q8(x, scale):
    return np.asarray(x * scale, np.float32).astype(E4NP)


def _split8(x, scale):
    hi = _q8(x, scale)
    lo = (np.asarray(x * scale, np.float32)
          - hi.astype(np.float32)).astype(E4NP)
    return hi, lo


def _pack_w_dr(Wmat, scale):
    """[Cdim, M] f32 -> (wa [128, ncb, 2, M] Wh-dup, wb [128, ncb/2, 2, M])."""
    Cdim, M = Wmat.shape
    ncb = Cdim // 128
    hi, lo = _split8(Wmat, scale)
    hi = hi.reshape(ncb, 128, M).transpose(1, 0, 2)   # [128, ncb, M]
    lo = lo.reshape(ncb, 128, M).transpose(1, 0, 2)
    wa = np.stack([hi, hi], axis=2)                   # [128, ncb, 2, M]
    wb = np.stack([lo[:, 0::2], lo[:, 1::2]], axis=2)  # [128, ncb/2, 2, M]
    return np.ascontiguousarray(wa), np.ascontiguousarray(wb)


def _pack_inputs(features, We1, be1, We2, be2, We3, be3,
                 Wd1, bd1, Wd2, bd2, Wd3, bd3, cat_idx, cap):
    """Dispatch rows to cores by category (expert-parallel sharding)."""
    features = np.asarray(features, np.float32)
    cat = np.asarray(cat_idx).astype(np.int64)
    order = np.argsort(cat, kind="stable")
    counts = np.bincount(cat, minlength=N_CORES)
    starts = np.zeros(N_CORES + 1, np.int64)
    np.cumsum(counts, out=starts[1:])

    def chunkcols(b):
        b = np.asarray(b, np.float32).reshape(-1)
        return b.reshape(-1, 128).T

    wa1, wb1 = _pack_w_dr(np.asarray(We1, np.float32), SW1)
    wa2, wb2 = _pack_w_dr(np.asarray(We2, np.float32), SW2)
    We3f = np.asarray(We3, np.float32)
    be3f = np.asarray(be3, np.float32)
    enc = dict(wa1=wa1, wb1=wb1, wa2=wa2, wb2=wb2)

    maps, rows_per_core = [], []
    for k in range(N_CORES):
        rows = order[starts[k]:starts[k + 1]]
        rows_per_core.append(rows)
        f = np.zeros((cap, C), np.float32)
        f[:len(rows)] = features[rows]
        fT = np.ascontiguousarray(f.T) * SF
        fhl = np.zeros((2, C, cap), E4NP)
        fhl[0] = fT.astype(E4NP)
        fhl[1] = (fT - fhl[0].astype(np.float32)).astype(E4NP)
        bias_all = np.zeros((128, NBIAS), np.float32)
        bias_all[:, OB1:OB1 + 4] = chunkcols(np.asarray(be1, np.float32) * SA1)
        bias_all[:, OB2:OB2 + 2] = chunkcols(be2)
        wd1k = np.asarray(Wd1, np.float32)[k]
        bias_all[:, OD1:OD1 + 2] = chunkcols(
            wd1k.T @ be3f + np.asarray(bd1, np.float32)[k])
        bias_all[:, OD2:OD2 + 2] = chunkcols(np.asarray(bd2, np.float32)[k])
        bias_all[:, OD3:OD3 + 1] = chunkcols(np.asarray(bd3, np.float32)[k])
        m = dict(enc)
        m["fhl"] = fhl
        m["wd1"] = We3f @ wd1k  # encoder L3 folded into decoder layer 1
        m["wd2"] = np.asarray(Wd2, np.float32)[k]
        m["wd3"] = np.asarray(Wd3, np.float32)[k]
        m["bias_all"] = bias_all
        maps.append(m)
    return maps, rows_per_core


_NC_CACHE = {}


def _get_nc(cap=4352):
    if cap not in _NC_CACHE:
        _NC_CACHE[cap] = _build_nc(cap)
    return _NC_CACHE[cap]


def kernel(**inputs) -> np.ndarray:
    cat = np.asarray(inputs["cat_idx"]).astype(np.int64)
    counts = np.bincount(cat, minlength=N_CORES)
    cap = max(256, int(-(-counts.max() // 256) * 256))
    maps, rows_per_core = _pack_inputs(**inputs, cap=cap)
    nc = _get_nc(cap)
    res = bass_utils.run_bass_kernel_spmd(nc, maps, core_ids=list(range(N_CORES)))
    latent = np.zeros((B, LAT), np.float32)
    for k, r in enumerate(res.results):
        rows = rows_per_core[k]
        latent[rows] = r["out"][:, :len(rows)].T
    return latent


# revision 47
# speedup vs baseline: 2.0366x; 2.0366x over previous
"""Trainium2 Bass kernel for nn_CategorySpecificInitNet (moe_routing).

kernel(**inputs) takes the FULL unsharded inputs (keys as in
reference.setup_inputs()) and returns the FULL [B, 128] float32 output.

Strategy — expert-parallel (dispatch-by-category), as in the proven
baseline, plus an fp8 DoubleRow fast path for the two big encoder
layers:

  - rows are stably sorted by cat_idx; core k gets category k's rows,
    zero-padded to a static capacity (max category count rounded up to
    256);
  - encoder L3 (256->256 linear) is constant-folded into decoder layer 1
    on the host (exact algebra);
  - L1 (768->512) and L2 (512->256) run as fp8e4m3 DoubleRow matmuls
    with full hi/lo error compensation: both weights and activations are
    split x ~= x_hi + x_lo (each e4m3), and y = W@x is computed as
    (Wh+Wl)@x_hi + Wh@x_lo (the Wl@x_lo term, ~0.15% rms, is dropped).
    DoubleRow processes two 128-contraction slots per instruction at
    half the per-row cost, so the 3-term compensated product runs at
    0.75x the fp32r cycle count while keeping end-to-end max rel error
    ~3e-3 (measured against the fp32 reference).
  - feature hi/lo splitting happens on the host (it is a dtype-level
    repacking of the input, like casting to bf16); a1's hi/lo pair is
    produced on device by ACT relu->fp8 (hi), DVE relu->f32, and a
    Pool/DVE subtract (lo), so no engine saturates.
  - the decoder (256->256->256->128) stays in float32r.
  - all compute is feature-major [features(partitions), rows(free)];
    per-core row tiles of 512 with 256-row drain tiles; the decoder is
    software-pipelined TWO tiles behind the encoder (d1 lags one tile,
    d2/d3 two), so every cross-engine dependency has a full matmul
    block of PE cover; the tensor engine is pre-warmed with dummy
    matmuls to beat the p-state ramp while the first DMAs land.
"""
import sys

for _p in ("/opt/trn_rl_repo",):
    if _p not in sys.path:
        sys.path.append(_p)

import numpy as np
import ml_dtypes

import concourse.bass as bass
import concourse.bacc as bacc
import concourse.mybir as mybir
import concourse.tile as tile
from concourse import bass_utils

FR = mybir.dt.float32r
F32 = mybir.dt.float32
F8 = mybir.dt.float8e4
E4NP = ml_dtypes.float8_e4m3
DR = mybir.MatmulPerfMode.DoubleRow
Alu = mybir.AluOpType
ActF = mybir.ActivationFunctionType

B, C, H1, H2, HO = 32768, 768, 512, 256, 256
DH, LAT, K = 256, 128, 8
N_CORES = 8
TILE = 512

# fp8 scales (powers of two).  features absmax ~5.1 -> x16 = ~81 (<240
# e4m3 max); a1 absmax ~2.8 -> x32 = ~90; weight absmax ~0.1 -> x1024.
SF = 16.0
SA1 = 32.0
SW1 = 1024.0
SW2 = 1024.0
A1_SCALE = SA1 / (SW1 * SF)       # psum1 -> a1-units scale for ACT
A2_SCALE = 1.0 / (SW2 * SA1)      # psum2 -> true a2 units

# bias_all columns: be1*SA1 [4] | be2 [2] | bf [2] | bd2 [2] | bd3 [1]
# | be1*SW1*SF [4] (psum-scale L1 bias for the DVE relu path)
OB1, OB2, OD1, OD2, OD3, OB1P = 0, 4, 6, 8, 10, 11
NBIAS = 15

nC, nH1, nH2, nDH = C // 128, H1 // 128, H2 // 128, DH // 128


def _build_nc(cap, tile_n=512, zero_bias=True):
    assert cap % 256 == 0
    # full 512 tiles, then 256-row drain tiles (short decoder tail)
    n512 = max(1, cap // tile_n)
    if cap % tile_n == 0:
        n512 -= 1
    tiles = [tile_n] * n512
    rem = cap - n512 * tile_n
    while rem:
        tiles.append(256)
        rem -= 256
    assert sum(tiles) == cap
    offs = [sum(tiles[:i]) for i in range(len(tiles))]
    nt = len(tiles)
    nc = bacc.Bacc(name="catnet_fp8")
    nc._phase_marks = []

    # fp8 hi/lo features, packed (2, C, cap): [0]=hi, [1]=lo
    fhl = nc.dram_tensor("fhl", (2, C, cap), F8, kind="ExternalInput")
    # L1 weights: wa1[p, c, 1, m] holds Wh once (the DoubleRow slot pair
    # (Wh,xh),(Wh,xl) reads it with a stride-0 broadcast);
    # wb1[p, cc, i, m]: (Wl_{2cc}, Wl_{2cc+1})
    wa1 = nc.dram_tensor("wa1", (128, nC, 1, H1), F8, kind="ExternalInput")
    wb1 = nc.dram_tensor("wb1", (128, nC // 2, 2, H1), F8, kind="ExternalInput")
    wa2 = nc.dram_tensor("wa2", (128, nH1, 1, H2), F8, kind="ExternalInput")
    wb2 = nc.dram_tensor("wb2", (128, nH1 // 2, 2, H2), F8, kind="ExternalInput")
    BF16 = mybir.dt.bfloat16
    wd1 = nc.dram_tensor("wd1", (H2, DH), FR, kind="ExternalInput")  # We3 @ Wd1
    wd2 = nc.dram_tensor("wd2", (DH, DH), FR, kind="ExternalInput")
    wd3 = nc.dram_tensor("wd3", (DH, LAT), FR, kind="ExternalInput")
    bias_all = nc.dram_tensor("bias_all", (128, NBIAS), F32, kind="ExternalInput")
    out = nc.dram_tensor("out", (LAT, cap), F32, kind="ExternalOutput")

    with tile.TileContext(nc) as tc:
        with (
            tc.tile_pool(name="wp", bufs=1) as wp,
            tc.tile_pool(name="fp", bufs=3) as fp,
            tc.tile_pool(name="xp", bufs=2) as xp,
            tc.tile_pool(name="ap", bufs=3) as ap,
            tc.tile_pool(name="dp", bufs=2) as dp,
            tc.tile_pool(name="ps_w", bufs=6, space="PSUM") as ps_w,
            tc.tile_pool(name="ps_o", bufs=2, space="PSUM") as ps_o,
        ):
            fhl_r = fhl.rearrange("t (c p) b -> p t c b", p=128)

            # tile-0 features first (sync queue), then L1 weights (gpsimd
            # queue) — the DMA engines serve roughly in arrival order, so
            # the critical-path transfers go in front.
            fx0_fl = fp.tile([128, 2, nC, tile_n], F8, tag="fx")
            fx0 = fx0_fl[:, :, :, :tiles[0]]
            nc.gpsimd.dma_start(fx0[:], fhl_r[:, :, :, :tiles[0]])
            wa1_t = wp.tile([128, nC, 1, H1], F8, tag="wa1")
            nc.sync.dma_start(wa1_t[:, :3], wa1[:, :3])
            nc.sync.dma_start(wa1_t[:, 3:], wa1[:, 3:])
            wb1_t = wp.tile([128, nC // 2, 2, H1], F8, tag="wb1")
            nc.sync.dma_start(wb1_t[:], wb1[:])
            bias_t = wp.tile([128, NBIAS], F32, tag="bias")
            nc.sync.dma_start(bias_t[:], bias_all[:])
            wa2_t = wp.tile([128, nH1, 1, H2], F8, tag="wa2")
            nc.sync.dma_start(wa2_t[:], wa2[:])
            wb2_t = wp.tile([128, nH1 // 2, 2, H2], F8, tag="wb2")
            nc.sync.dma_start(wb2_t[:], wb2[:])
            wd1_t = wp.tile([128, nH2, DH], FR, tag="wd1")
            wd2_t = wp.tile([128, nDH, DH], FR, tag="wd2")
            wd3_t = wp.tile([128, nDH, LAT], FR, tag="wd3")

            def fetch_wd():
                nc.sync.dma_start(wd1_t[:], wd1.rearrange("(c p) d -> p c d", p=128))
                nc.sync.dma_start(wd2_t[:], wd2.rearrange("(c p) d -> p c d", p=128))
                nc.sync.dma_start(wd3_t[:], wd3.rearrange("(c p) d -> p c d", p=128))

            # PE p-state warmup: the tensor engine ramps to full clock only
            # after ~3us of continuous activity, so burn cheap matmuls on a
            # zeroed tile while the first feature DMA is in flight.
            wz = wp.tile([128, 512], BF16, tag="warmzero")
            nc.vector.memset(wz[:], 0.0)
            pwarm = ps_o.tile([128, tile_n], F32, tag="out")
            for _ in range(16):
                nc.tensor.matmul(pwarm[:, :256], wz[:, :128], wz[:, :256],
                                 start=True, stop=True)

            def fp8_layer3t(wa_t, wb_t, x_t, ncb, m, tn, pw, defer_last=False,
                            wb_last=False):
                """3-term compensated fp8 DoubleRow matmul accumulation.

                wa_t: [128, ncb, 2, M] (Wh dup), wb_t: [128, ncb/2, 2, M]
                x_t:  [128, 2, ncb, tn] hi/lo pair
                accumulates (Wh+Wl)@xh + Wh@xl into pw[:, :tn].

                The (Wh, hi+lo) instruction of the LAST c-block touches the
                freshest lo activation; with defer_last the caller gets it
                back as closures to weave behind independent PE work.
                """
                msl = bass.ts(m, 128)
                last = []
                for ch in range(0, tn, 256):
                    n = min(256, tn - ch)
                    sl = slice(ch, ch + n)
                    if wb_last:
                        # all Wh instrs first (weights land before Wl at
                        # startup), Wl instrs carry the stop flag
                        for c in range(ncb):
                            nc.tensor.matmul(
                                pw[:, sl],
                                wa_t[:, c, :, msl].broadcast_to((128, 2, 128)),
                                x_t[:, :, c, sl],
                                start=(c == 0), stop=False, perf_mode=DR)
                        for cc in range(ncb // 2):
                            nc.tensor.matmul(
                                pw[:, sl], wb_t[:, cc, :, msl],
                                x_t[:, 0, 2 * cc:2 * cc + 2, sl],
                                start=False, stop=(cc == ncb // 2 - 1),
                                perf_mode=DR)
                        continue
                    first = True
                    for c in range(ncb - 1):
                        nc.tensor.matmul(
                            pw[:, sl],
                            wa_t[:, c, :, msl].broadcast_to((128, 2, 128)),
                            x_t[:, :, c, sl],
                            start=first, stop=False, perf_mode=DR)
                        first = False
                    for cc in range(ncb // 2):
                        nc.tensor.matmul(
                            pw[:, sl], wb_t[:, cc, :, msl],
                            x_t[:, 0, 2 * cc:2 * cc + 2, sl],
                            start=False, stop=False, perf_mode=DR)

                    def fin(sl=sl):
                        nc.tensor.matmul(
                            pw[:, sl],
                            wa_t[:, ncb - 1, :, msl].broadcast_to((128, 2, 128)),
                            x_t[:, :, ncb - 1, sl],
                            start=False, stop=True, perf_mode=DR)
                    if defer_last:
                        last.append(fin)
                    else:
                        fin()
                return last

            def fetch_fx(t):
                tn = tiles[t]
                sl = slice(offs[t], offs[t] + tn)
                fx_fl = fp.tile([128, 2, nC, tile_n], F8, tag="fx")
                fx = fx_fl[:, :, :, :tn]
                nc.sync.dma_start(fx[:], fhl_r[:, :, :, sl])
                return fx

            def emit_l1(t, fx):
                tn = tiles[t]
                # L1: 768 -> 512, fp8 3-term.  a1 hi/lo pairs per m-block:
                #   hi  = ACT  relu(psum*scale + b)        -> fp8 (from PSUM)
                #   x32 = DVE  relu(psum + b/scale)        -> f32 (psum units)
                #   lo  = DVE  x32*scale - hi (stt)        -> fp8
                # ACT and DVE read the psum in parallel, so the pair is
                # ready ~1.2us after the m-block's matmuls finish.
                x32_fl = xp.tile([128, nH1, tile_n], F32, tag="x32")
                x32 = x32_fl[:, :, :tn]
                a1_fl = xp.tile([128, 2, nH1, tile_n], F8, tag="a1")
                a1 = a1_fl[:, :, :, :tn]
                for m in range(nH1):
                    pw_fl = ps_w.tile([128, tile_n], F32, tag="pw")
                    pw = pw_fl[:, :tn]
                    fp8_layer3t(wa1_t, wb1_t, fx, nC, m, tn, pw_fl,
                                wb_last=True)
                    nc.scalar.activation(a1[:, 0, m, :], pw[:], ActF.Relu,
                                         bias=bias_t[:, OB1 + m:OB1 + m + 1],
                                         scale=A1_SCALE)
                    if zero_bias:
                        # x32 = relu(psum*scale) in a1 units; lo = x32 - hi
                        # (plain subtract, which GPSIMD supports)
                        nc.vector.tensor_scalar(x32[:, m, :], pw[:],
                                                A1_SCALE, 0.0,
                                                Alu.mult, Alu.max)
                        eng = nc.gpsimd if m < 2 else nc.vector
                        eng.tensor_tensor(a1[:, 1, m, :], x32[:, m, :],
                                          a1[:, 0, m, :], Alu.subtract)
                    else:
                        # generic bias: x32 in psum units, fold the scale
                        # into the DVE lo op
                        nc.vector.tensor_scalar(x32[:, m, :], pw[:],
                                                bias_t[:, OB1P + m:OB1P + m + 1],
                                                0.0, Alu.add, Alu.max)
                        nc.vector.scalar_tensor_tensor(
                            a1[:, 1, m, :], x32[:, m, :], A1_SCALE,
                            a1[:, 0, m, :], Alu.mult, Alu.subtract)
                return a1

            def emit_l2_m(t, a1, m, defer_last):
                tn = tiles[t]
                pw_fl = ps_w.tile([128, tile_n], F32, tag="pw")
                pw = pw_fl[:, :tn]
                last = fp8_layer3t(wa2_t, wb2_t, a1, nH1, m, tn, pw_fl,
                                   defer_last=defer_last)

                def act():
                    x_fl = ap.tile([128, tile_n], FR, tag=f"a2_{m}")
                    x = x_fl[:, :tn]
                    nc.scalar.activation(x[:], pw[:], ActF.Relu,
                                         bias=bias_t[:, OB2 + m:OB2 + m + 1],
                                         scale=A2_SCALE)
                    return x
                return last, act

            def emit_d1(t, h):
                tn = tiles[t]
                pws, d1 = [], []
                for m in range(nDH):
                    pw_fl = ps_w.tile([128, tile_n], F32, tag="pw")
                    pws.append(pw_fl[:, :tn])
                for c in range(nH2):            # c-major: a2_m1 arrives late
                    for m in range(nDH):
                        nc.tensor.matmul(pws[m], wd1_t[:, c, bass.ts(m, 128)],
                                         h[c][:],
                                         start=(c == 0), stop=(c == nH2 - 1))
                for m in range(nDH):
                    x_fl = dp.tile([128, tile_n], FR, tag=f"d1_{m}")
                    x = x_fl[:, :tn]
                    if m % 2 == 1:
                        nc.vector.tensor_scalar(x[:], pws[m],
                                                bias_t[:, OD1 + m:OD1 + m + 1],
                                                0.0, Alu.add, Alu.max)
                    else:
                        nc.scalar.activation(x[:], pws[m], ActF.Relu,
                                             bias=bias_t[:, OD1 + m:OD1 + m + 1])
                    d1.append(x)
                return d1

            def emit_d2(t, d1):
                tn = tiles[t]
                pws, d2 = [], []
                for m in range(nDH):
                    pw_fl = ps_w.tile([128, tile_n], F32, tag="pw")
                    pws.append(pw_fl[:, :tn])
                for c in range(nDH):
                    for m in range(nDH):
                        nc.tensor.matmul(pws[m], wd2_t[:, c, bass.ts(m, 128)],
                                         d1[c][:],
                                         start=(c == 0), stop=(c == nDH - 1))

                def acts(split_engines=False):
                    for m in range(nDH):
                        x_fl = dp.tile([128, tile_n], FR, tag=f"d2_{m}")
                        x = x_fl[:, :tn]
                        bb = bias_t[:, OD2 + m:OD2 + m + 1]
                        if split_engines and m % 2 == 1:
                            nc.vector.tensor_scalar(x[:], pws[m], bb, 0.0,
                                                    Alu.add, Alu.max)
                        else:
                            nc.scalar.activation(x[:], pws[m], ActF.Relu, bias=bb)
                        d2.append(x)
                    return d2
                return acts

            def emit_d3_mm(t, d2, po, c):
                nc.tensor.matmul(po, wd3_t[:, c, :], d2[c][:],
                                 start=(c == 0), stop=(c == nDH - 1))

            def emit_d3_store(t, po):
                tn = tiles[t]
                osb_fl = ap.tile([128, tile_n], F32, tag="osb")
                osb = osb_fl[:, :tn]
                nc.scalar.activation(osb[:], po[:], ActF.Identity,
                                     bias=bias_t[:, OD3:OD3 + 1])
                nc.sync.dma_start(out[:, offs[t]:offs[t] + tn], osb[:])

            def emit_d2_d3_store(tp, d1p, split=False):
                d2acts = emit_d2(tp, d1p)
                if not split:
                    d2p = d2acts()
                    po_fl = ps_o.tile([128, tile_n], F32, tag="out")
                    po = po_fl[:, :tiles[tp]]
                    emit_d3_mm(tp, d2p, po, 0)
                    emit_d3_mm(tp, d2p, po, 1)
                    emit_d3_store(tp, po)
                    return
                # drain tile: column-halved d3 + store; the two halves
                # use independent engines/queues (ACT+SP vs DVE+SWDGE) so
                # the final store chains fully overlap
                tn = tiles[tp]
                d2p = d2acts(split_engines=True)
                po_fl = ps_o.tile([128, tile_n], F32, tag="out")
                osb_fl = ap.tile([128, tile_n], F32, tag="osb")
                h = tn // 2
                for i, s in enumerate((slice(0, h), slice(h, tn))):
                    po = po_fl[:, s]
                    for c in range(nDH):
                        nc.tensor.matmul(po, wd3_t[:, c, :], d2p[c][:, s],
                                         start=(c == 0), stop=(c == nDH - 1))
                    if i == 0:
                        nc.scalar.activation(osb_fl[:, s], po, ActF.Identity,
                                             bias=bias_t[:, OD3:OD3 + 1])
                        nc.sync.dma_start(
                            out[:, offs[tp] + s.start:offs[tp] + s.stop],
                            osb_fl[:, s])
                    else:
                        nc.vector.tensor_scalar(osb_fl[:, s], po,
                                                bias_t[:, OD3:OD3 + 1], 0.0,
                                                Alu.add, Alu.add)
                        nc.sync.dma_start(
                            out[:, offs[tp] + s.start:offs[tp] + s.stop],
                            osb_fl[:, s])

            def mark(label):
                nc._phase_marks.append(
                    (int(nc.get_next_instruction_name()[2:]), label))

            # Two-deep software pipeline.  PE stream per step t:
            #   L1(t) | d1(t-1) | d2mm(t-2) | L2 mains(t) | d3c0(t-2) |
            #   L2 fins(t) | d3c1(t-2) | store(t-2)
            # Every cross-engine dependency gets at least a full matmul
            # block of PE cover, so ACT/DVE/Pool latency never stalls PE.
            a2s, d1s, a1s = {}, {}, {}
            fx_cur = fx0
            fx_nn = None
            for t in range(nt):
                if t > 0:
                    mark(f"fetch{t + 1}")
                    fx_next = fx_nn
                    fx_nn = fetch_fx(t + 2) if t + 2 < nt else None
                mark(f"L1.{t}")
                a1 = emit_l1(t, fx_cur)
                if t == 0:
                    # issue after L1(0) so its matmuls don't inherit waits
                    # on these later transfers (queue-coarse semaphores)
                    mark("fetch12")
                    fx_next = fetch_fx(1) if nt > 1 else None
                    fx_nn = fetch_fx(2) if nt > 2 else None
                    fetch_wd()
                fx_cur = fx_next
                if t == 0:
                    # pipeline fill: L2(0) runs in step 1 under L1(1) cover
                    a1s[0] = a1
                    continue
                if t == 1:
                    a1p = a1s.pop(0)
                    mark("L2.0")
                    _, act0 = emit_l2_m(0, a1p, 0, defer_last=False)
                    _, act1 = emit_l2_m(0, a1p, 1, defer_last=False)
                    a2s[0] = [act0(), act1()]
                mark(f"d1.{t - 1}")
                d1s[t - 1] = emit_d1(t - 1, a2s.pop(t - 1))
                d2acts = emit_d2(t - 2, d1s.pop(t - 2)) if t >= 2 else None
                if d2acts is not None:
                    d2p = d2acts()
                    po_fl = ps_o.tile([128, tile_n], F32, tag="out")
                    po = po_fl[:, :tiles[t - 2]]
                    mark(f"d3.{t - 2}")
                    emit_d3_mm(t - 2, d2p, po, 0)
                mark(f"L2.{t}")
                _, l2act0 = emit_l2_m(t, a1, 0, defer_last=False)
                h0 = l2act0()
                if d2acts is not None:
                    mark(f"d3b.{t - 2}")
                    emit_d3_mm(t - 2, d2p, po, 1)
                _, l2act1 = emit_l2_m(t, a1, 1, defer_last=False)
                h1 = l2act1()
                if d2acts is not None:
                    emit_d3_store(t - 2, po)
                mark(f"a2acts.{t}")
                a2s[t] = [h0, h1]
            mark("drain")
            # drain: d2mm(nt-2) | d1(nt-1) fills the d2-ACT wait |
            # d3(nt-2)+store | d2/d3/store(nt-1)
            if nt >= 2:
                d2acts_p = emit_d2(nt - 2, d1s.pop(nt - 2))
                d1_last = emit_d1(nt - 1, a2s.pop(nt - 1))
                d2p_p = d2acts_p()
                po_fl = ps_o.tile([128, tile_n], F32, tag="out")
                po = po_fl[:, :tiles[nt - 2]]
                emit_d3_mm(nt - 2, d2p_p, po, 0)
                emit_d3_mm(nt - 2, d2p_p, po, 1)
                emit_d3_store(nt - 2, po)
            else:
                d1_last = emit_d1(nt - 1, a2s.pop(nt - 1))
            emit_d2_d3_store(nt - 1, d1_last, split=True)

    nc.finalize()
    return nc


def _q8(x, scale):
    return np.asarray(x * scale, np.float32).astype(E4NP)


def _split8(x, scale):
    hi = _q8(x, scale)
    lo = (np.asarray(x * scale, np.float32)
          - hi.astype(np.float32)).astype(E4NP)
    return hi, lo


def _pack_w_dr(Wmat, scale):
    """[Cdim, M] f32 -> (wa [128, ncb, 1, M] Wh, wb [128, ncb/2, 2, M] Wl)."""
    Cdim, M = Wmat.shape
    ncb = Cdim // 128
    hi, lo = _split8(Wmat, scale)
    hi = hi.reshape(ncb, 128, M).transpose(1, 0, 2)   # [128, ncb, M]
    lo = lo.reshape(ncb, 128, M).transpose(1, 0, 2)
    wa = hi[:, :, None, :]                            # [128, ncb, 1, M]
    wb = np.stack([lo[:, 0::2], lo[:, 1::2]], axis=2)  # [128, ncb/2, 2, M]
    return np.ascontiguousarray(wa), np.ascontiguousarray(wb)


def _pack_inputs(features, We1, be1, We2, be2, We3, be3,
                 Wd1, bd1, Wd2, bd2, Wd3, bd3, cat_idx, cap):
    """Dispatch rows to cores by category (expert-parallel sharding)."""
    features = np.asarray(features, np.float32)
    cat = np.asarray(cat_idx).astype(np.int64)
    order = np.argsort(cat, kind="stable")
    counts = np.bincount(cat, minlength=N_CORES)
    starts = np.zeros(N_CORES + 1, np.int64)
    np.cumsum(counts, out=starts[1:])

    def chunkcols(b):
        b = np.asarray(b, np.float32).reshape(-1)
        return b.reshape(-1, 128).T

    wa1, wb1 = _pack_w_dr(np.asarray(We1, np.float32), SW1)
    wa2, wb2 = _pack_w_dr(np.asarray(We2, np.float32), SW2)
    We3f = np.asarray(We3, np.float32)
    be3f = np.asarray(be3, np.float32)
    enc = dict(wa1=wa1, wb1=wb1, wa2=wa2, wb2=wb2)

    maps, rows_per_core = [], []
    for k in range(N_CORES):
        rows = order[starts[k]:starts[k + 1]]
        rows_per_core.append(rows)
        f = np.zeros((cap, C), np.float32)
        f[:len(rows)] = features[rows]
        fT = np.ascontiguousarray(f.T) * SF
        fhl = np.zeros((2, C, cap), E4NP)
        fhl[0] = fT.astype(E4NP)
        fhl[1] = (fT - fhl[0].astype(np.float32)).astype(E4NP)
        bias_all = np.zeros((128, NBIAS), np.float32)
        bias_all[:, OB1:OB1 + 4] = chunkcols(np.asarray(be1, np.float32) * SA1)
        bias_all[:, OB1P:OB1P + 4] = chunkcols(
            np.asarray(be1, np.float32) * (SW1 * SF))
        bias_all[:, OB2:OB2 + 2] = chunkcols(be2)
        wd1k = np.asarray(Wd1, np.float32)[k]
        bias_all[:, OD1:OD1 + 2] = chunkcols(
            wd1k.T @ be3f + np.asarray(bd1, np.float32)[k])
        bias_all[:, OD2:OD2 + 2] = chunkcols(np.asarray(bd2, np.float32)[k])
        bias_all[:, OD3:OD3 + 1] = chunkcols(np.asarray(bd3, np.float32)[k])
        m = dict(enc)
        m["fhl"] = fhl
        m["wd1"] = We3f @ wd1k  # encoder L3 folded into decoder layer 1
        m["wd2"] = np.asarray(Wd2, np.float32)[k]
        m["wd3"] = np.asarray(Wd3, np.float32)[k]
        m["bias_all"] = bias_all
        maps.append(m)
    return maps, rows_per_core


_NC_CACHE = {}


def _get_nc(cap=4352, zero_bias=True):
    key = (cap, zero_bias)
    if key not in _NC_CACHE:
        _NC_CACHE[key] = _build_nc(cap, zero_bias=zero_bias)
    return _NC_CACHE[key]


def kernel(**inputs) -> np.ndarray:
    cat = np.asarray(inputs["cat_idx"]).astype(np.int64)
    counts = np.bincount(cat, minlength=N_CORES)
    cap = max(512, int(-(-counts.max() // 256) * 256))
    maps, rows_per_core = _pack_inputs(**inputs, cap=cap)
    zb = all(not np.any(np.asarray(inputs[k]))
             for k in ("be1", "be2", "be3", "bd1", "bd2", "bd3"))
    nc = _get_nc(cap, zero_bias=zb)
    res = bass_utils.run_bass_kernel_spmd(nc, maps, core_ids=list(range(N_CORES)))
    latent = np.zeros((B, LAT), np.float32)
    for k, r in enumerate(res.results):
        rows = rows_per_core[k]
        latent[rows] = r["out"][:, :len(rows)].T
    return latent
